# revision 1
# baseline (speedup 1.0000x reference)
"""Trainium2 Bass kernel for the AngularCosDiff (ANI-style angular symmetry
function) problem.

out[p, a*9+z] = 4 * exp(-(Gamma_z*(cos_p - cos(ShfZ_z))^2
                          + EtaA*(0.5*(d1_p+d2_p) - ShfA_a)^2)) * fcj1_p*fcj2_p

Data-parallel over the pair dimension P across 8 NeuronCores; the small
constant vectors are folded on the host into per-partition scale/bias
operands for the ScalarEngine's fused Square(scale*x+bias) ops.

All transcendentals use the single `natural_log_exp_and_others` ACT table
set (exp/ln/square): sqrt(x) = exp(0.5*ln x), 1/(d1*d2) = exp(-0.5*(l1+l2)).
"""

import math

import numpy as np

import concourse.bass as bass
import concourse.bacc as bacc
import concourse.mybir as mybir
from concourse.tile import TileContext
from concourse.bass_utils import run_bass_kernel_spmd

F32 = mybir.dt.float32
AF = mybir.ActivationFunctionType


def _patch_act_tables():
    """Make {square, ln, exp} resolve to the single set
    `natural_log_exp_and_others` so bacc's table-load pass emits one
    LoadActFuncSet instead of thrashing between sets on every Square/Ln/Exp
    boundary (~2.7us per reload). Set order/indices are preserved; only the
    membership used by the load-insertion analysis is filtered.
    """
    import concourse.hw_specs as hw_specs

    if getattr(hw_specs, "_angular_patch", False):
        return
    orig = hw_specs.get_activation_tables

    def patched(module_arch):
        tabs = orig(module_arch)
        ours = {
            AF.Square,
            AF.Ln,
            AF.Exp,
            AF.Identity,
            AF.Copy,
        }
        out = {}
        for name, fns in tabs.items():
            if name == "natural_log_exp_and_others":
                out[name] = fns
            else:
                out[name] = {fn for fn in fns if fn not in ours}
        return out

    hw_specs.get_activation_tables = patched
    # bacc imports the symbol directly
    import concourse.bacc as _bacc_mod

    if hasattr(_bacc_mod, "get_activation_tables"):
        _bacc_mod.get_activation_tables = patched
    hw_specs._angular_patch = True

N_CORES = 8
P_TOTAL = 4_194_304
PC = P_TOTAL // N_CORES          # pairs per core
CUTOFF = 3.5
C2 = CUTOFF * CUTOFF
A_DIM = 4
Z_DIM = 9
OUT_D = A_DIM * Z_DIM            # 36 (E=1)

F = 512                          # pairs per partition per tile
N_OUT_CHUNKS = 4                 # output store granularity (F must divide)
INPLACE = True                   # reuse tiles in place to fit F=512 in SBUF
VIN_BUFS = 3
ARGS_BUFS = 2
OUTC_BUFS = 5
DLS_PSUM = False                 # put the [d1 d2 l1 l2] tile in PSUM
NSPLIT = 1                       # f-slices for the squares/exp stage
OUT_BF16 = 1                     # store output as bf16 (halves out-DMA bytes)
SMALL_BUFS = 2                   # bufs for the small chain tiles
GPS_FRONT = 0                    # run small front-end DVE ops on GpSimd
GPS_E2P = 0                      # run the e2p multiply on GpSimd
GPS_M = 0                        # run the v1*v2 multiply on GpSimd
GPS_OUTER = 0                    # chunks with index < GPS_OUTER run on GpSimd
MERGE_IN_DMA = 0                 # load v1+v2 with a single DMA
PIPE_DEPTH = 2                   # how many fronts run ahead of each back
OUTER4D = 0                      # one 4-dim-AP outer instr per chunk
TAPER = 2                        # split first/last tiles in half (ramp/drain)
RV_PSUM = 0                      # place rv (1/(d1*d2)) in PSUM
COST_PSUM = 0                    # place the cos(angle) tile in PSUM
FCOS = 0                         # fuse fc and cos into one TT via mixed tile


def build_nc(pc: int = PC, f: int = F, repeat: int = 1):
    """Build the per-core Bass program for a shard of `pc` pairs.

    Emission is software-pipelined: tile i+1's front-end (loads through
    exp of the gaussian args) is emitted before tile i's back-end (outer
    product + store), so the scheduler keeps the ACT chain of the next
    tile running while the DVE outer block of the current tile drains.

    repeat>1 builds a benchmarking variant: the whole program body runs
    `repeat` times into an internal DRAM buffer (tiny external token
    output) so device time can be measured by differencing wall times.
    """
    _patch_act_tables()
    assert pc % (128 * f) == 0
    ntiles = pc // (128 * f)
    fq = f // N_OUT_CHUNKS

    nc = bacc.Bacc("TRN2", target_bir_lowering=False, debug=False)

    v12 = nc.declare_dram_parameter("vectors12", [2, pc, 3], F32, isOutput=False)
    rscale_d = nc.declare_dram_parameter("rscale", [128, Z_DIM], F32, isOutput=False)
    rbias_d = nc.declare_dram_parameter("rbias", [128, Z_DIM], F32, isOutput=False)
    qscale_d = nc.declare_dram_parameter("qscale", [128, 1], F32, isOutput=False)
    qbias_d = nc.declare_dram_parameter("qbias", [128, A_DIM], F32, isOutput=False)
    fbias_d = nc.declare_dram_parameter("fbias", [128, 1], F32, isOutput=False)
    out_dt = mybir.dt.bfloat16 if OUT_BF16 else F32
    if repeat == 1:
        out_d = nc.declare_dram_parameter("out", [pc, OUT_D], out_dt, isOutput=True)
    else:
        out_d = nc.dram_tensor("out_scratch", [pc, OUT_D], out_dt)
        tok_d = nc.declare_dram_parameter("tok", [128, 1], F32, isOutput=True)

    s2c = math.sqrt(2.0) / C2     # Square(s2c*d2 - sqrt2) = 2*(C2-d^2)^2/C2^2

    with TileContext(nc) as tc:
        with tc.tile_pool(name="consts", bufs=1) as cpool:
            rs = cpool.tile([128, Z_DIM], F32, name="rs")
            rb = cpool.tile([128, Z_DIM], F32, name="rb")
            qs = cpool.tile([128, 1], F32, name="qs")
            qb = cpool.tile([128, A_DIM], F32, name="qb")
            fb = cpool.tile([128, 1], F32, name="fb")
            # const loads ride the ACT HWDGE ring so they don't sit ahead
            # of tile 0's input DMAs in the sync-ring FIFO
            nc.scalar.dma_start(rs, rscale_d.ap())
            nc.scalar.dma_start(rb, rbias_d.ap())
            nc.scalar.dma_start(qs, qscale_d.ap())
            nc.scalar.dma_start(qb, qbias_d.ap())
            nc.scalar.dma_start(fb, fbias_d.ap())

            with (
                tc.tile_pool(name="work", bufs=1) as pool,
                tc.tile_pool(name="psumw", bufs=1, space="PSUM") as ppool,
            ):

                def front(i, base, f):
                    """Loads + per-pair chain through exp(-args). Returns
                    the tiles the back-end needs."""
                    fq = f // N_OUT_CHUNKS
                    vin = pool.tile(
                        [128, (9 if INPLACE == 2 else 6) * f], F32, tag="vin",
                        bufs=VIN_BUFS, name=f"vin{i}"
                    )
                    if MERGE_IN_DMA:
                        nc.sync.dma_start(
                            vin[:, 0 : 6 * f].rearrange("p (j g) -> p j g", j=2),
                            bass.AP(
                                v12, 3 * base,
                                [[3 * f, 128], [3 * pc, 2], [1, 3 * f]],
                            ),
                        )
                    else:
                        nc.sync.dma_start(
                            vin[:, 0 : 3 * f],
                            bass.AP(v12, 3 * base, [[3 * f, 128], [1, 3 * f]]),
                        )
                        nc.sync.dma_start(
                            vin[:, 3 * f : 6 * f],
                            bass.AP(v12, 3 * (pc + base), [[3 * f, 128], [1, 3 * f]]),
                        )

                    # m = v1*v2 first (square clobbers vin), then square vin
                    # in place; sum xyz groups into ddd = [d1sq | d2sq | dot]
                    assert INPLACE or not FCOS
                    assert INPLACE != 2 or not FCOS
                    ddd = pool.tile(
                        [128, (2 if FCOS else 3) * f], F32, tag="ddd",
                        bufs=SMALL_BUFS, name=f"ddd{i}"
                    )
                    if INPLACE == 2:
                        # vin = [v1 | v2 | v1*v2]; square v1,v2 in place, then
                        # two merged 3-block adds give [d1sq | d2sq | dot]
                        nc.vector.tensor_mul(
                            vin[:, 6 * f : 9 * f],
                            vin[:, 0 : 3 * f],
                            vin[:, 3 * f : 6 * f],
                        )
                        nc.scalar.activation(
                            vin[:, 0 : 6 * f], vin[:, 0 : 6 * f], AF.Square
                        )
                        vin4 = vin.rearrange("p (j f c) -> p j f c", j=3, f=f, c=3)
                        ddd3 = ddd.rearrange("p (j f) -> p j f", j=3)
                        nc.vector.tensor_add(
                            ddd3, vin4[:, :, :, 0], vin4[:, :, :, 1]
                        )
                        nc.vector.tensor_add(ddd3, ddd3, vin4[:, :, :, 2])
                    if FCOS:
                        # tmix = [ft1 | dot | ft2 | rv]; one TT then yields
                        # [fc | cos] = tmix[0:2f] * tmix[2f:4f]
                        tmix = pool.tile([128, 4 * f], F32, tag="tmix",
                                         bufs=SMALL_BUFS, name=f"tmix{i}")
                    eng = nc.gpsimd if GPS_FRONT else nc.vector
                    if INPLACE == 2:
                        pass
                    elif INPLACE:
                        m = pool.tile([128, 3 * f], F32, tag="m", bufs=1,
                                      name=f"m{i}")
                        meng = nc.gpsimd if GPS_M else nc.vector
                        meng.tensor_mul(
                            m, vin[:, 0 : 3 * f], vin[:, 3 * f : 6 * f]
                        )
                        nc.scalar.activation(vin, vin, AF.Square)
                        vin4 = vin.rearrange("p (j f c) -> p j f c", j=2, f=f, c=3)
                        dd2 = ddd[:, 0 : 2 * f].rearrange("p (j f) -> p j f", j=2)
                        eng.tensor_add(dd2, vin4[:, :, :, 0], vin4[:, :, :, 1])
                        eng.tensor_add(dd2, dd2, vin4[:, :, :, 2])
                        m3 = m.rearrange("p (f c) -> p f c", c=3)
                        dot = tmix[:, f : 2 * f] if FCOS else ddd[:, 2 * f : 3 * f]
                        eng.tensor_add(dot, m3[:, :, 0], m3[:, :, 1])
                        eng.tensor_add(dot, dot, m3[:, :, 2])
                    else:
                        # sqm = [v1*v1 | v2*v2 | v1*v2]; 2 merged 3-block adds
                        sqm = pool.tile([128, 9 * f], F32, tag="sqm", bufs=2,
                                        name=f"sqm{i}")
                        nc.scalar.activation(sqm[:, 0 : 6 * f], vin, AF.Square)
                        nc.vector.tensor_mul(
                            sqm[:, 6 * f : 9 * f],
                            vin[:, 0 : 3 * f],
                            vin[:, 3 * f : 6 * f],
                        )
                        sqm4 = sqm.rearrange("p (j f c) -> p j f c", j=3, f=f, c=3)
                        ddd3 = ddd.rearrange("p (j f) -> p j f", j=3)
                        eng.tensor_add(ddd3, sqm4[:, :, :, 0], sqm4[:, :, :, 1])
                        eng.tensor_add(ddd3, ddd3, sqm4[:, :, :, 2])

                    # dls = [d1 | d2 | l1 | l2]; l = ln(dsq), d = exp(0.5*l)
                    dpool = ppool if DLS_PSUM else pool
                    dls = dpool.tile([128, 4 * f], F32, tag="dls", bufs=SMALL_BUFS,
                                     name=f"dls{i}")
                    nc.scalar.activation(dls[:, 2 * f : 4 * f], ddd[:, 0 : 2 * f], AF.Ln)
                    nc.scalar.activation(
                        dls[:, 0 : 2 * f], dls[:, 2 * f : 4 * f], AF.Exp, scale=0.5
                    )

                    # ft = [2*fcj1 | 2*fcj2] = Square(s2c*dsq - sqrt2)
                    if FCOS:
                        ft_out = tmix.rearrange(
                            "p (j g) -> p j g", j=2, g=2 * f
                        )[:, :, 0:f]
                        nc.scalar.activation(
                            ft_out, ddd.rearrange("p (j f) -> p j f", j=2),
                            AF.Square, scale=s2c, bias=fb[:, 0:1],
                        )
                    else:
                        ft = pool.tile([128, 2 * f], F32, tag="ft",
                                       bufs=SMALL_BUFS, name=f"ft{i}")
                        nc.scalar.activation(
                            ft, ddd[:, 0 : 2 * f], AF.Square, scale=s2c,
                            bias=fb[:, 0:1],
                        )

                    # slp = [d1+d2 | l1+l2]
                    slp = pool.tile([128, 2 * f], F32, tag="slp", bufs=SMALL_BUFS,
                                    name=f"slp{i}")
                    slp2 = slp.rearrange("p (j f) -> p j f", j=2)
                    dls2 = dls.rearrange("p (j g) -> p j g", j=2, g=2 * f)
                    eng.tensor_add(slp2, dls2[:, :, 0:f], dls2[:, :, f : 2 * f])

                    # rv = 1/(d1*d2) = exp(-0.5*(l1+l2))
                    if FCOS:
                        nc.scalar.activation(
                            tmix[:, 3 * f : 4 * f], slp[:, f : 2 * f],
                            AF.Exp, scale=-0.5,
                        )
                        # [fc | cos] = [ft1 | dot] * [ft2 | rv]
                        fcos = pool.tile([128, 2 * f], F32, tag="fcos",
                                         bufs=SMALL_BUFS, name=f"fcos{i}")
                        eng.tensor_mul(
                            fcos, tmix[:, 0 : 2 * f], tmix[:, 2 * f : 4 * f]
                        )
                        fc = fcos[:, 0:f]
                        cost = fcos[:, f : 2 * f]
                    else:
                        rv = (ppool if RV_PSUM else pool).tile(
                            [128, f], F32, tag="rv", bufs=SMALL_BUFS,
                            name=f"rv{i}"
                        )
                        nc.scalar.activation(
                            rv, slp[:, f : 2 * f], AF.Exp, scale=-0.5
                        )
                        # fc = 4*fcj1*fcj2 ; cost = cos(angle)
                        fc = pool.tile([128, f], F32, tag="fc", bufs=1,
                                       name=f"fc{i}")
                        eng.tensor_mul(fc, ft[:, 0:f], ft[:, f : 2 * f])
                        cost = (ppool if COST_PSUM else pool).tile(
                            [128, f], F32, tag="cost", bufs=SMALL_BUFS,
                            name=f"cost{i}"
                        )
                        eng.tensor_mul(cost, ddd[:, 2 * f : 3 * f], rv)

                    # 13 gaussian args: Square(scale*x + bias), computed in
                    # NSPLIT f-slices so exp/outer of slice 0 can start while
                    # the squares of slice 1 still run; q block first
                    args = pool.tile([128, 13 * f], F32, tag="args", bufs=ARGS_BUFS,
                                     name=f"args{i}")
                    fh = f // NSPLIT
                    args3q = args[:, Z_DIM * f : 13 * f].rearrange(
                        "p (a f) -> p a f", a=A_DIM
                    )
                    args3r = args[:, 0 : Z_DIM * f].rearrange(
                        "p (z f) -> p z f", z=Z_DIM
                    )
                    for h in range(NSPLIT):
                        lo, hi = h * fh, (h + 1) * fh
                        for a in range(A_DIM):
                            nc.scalar.activation(
                                args3q[:, a, lo:hi],
                                slp[:, lo:hi],
                                AF.Square,
                                scale=qs[:, 0:1],
                                bias=qb[:, a : a + 1],
                            )
                        for z in range(Z_DIM):
                            nc.scalar.activation(
                                args3r[:, z, lo:hi],
                                cost[:, lo:hi],
                                AF.Square,
                                scale=rs[:, z : z + 1],
                                bias=rb[:, z : z + 1],
                            )
                        # exp(-args) in place; q part first (feeds e2p)
                        nc.scalar.activation(
                            args3q[:, :, lo:hi], args3q[:, :, lo:hi],
                            AF.Exp, scale=-1.0,
                        )
                        nc.scalar.activation(
                            args3r[:, :, lo:hi], args3r[:, :, lo:hi],
                            AF.Exp, scale=-1.0,
                        )
                    return {"args": args, "fc": fc, "base": base, "f": f}

                def back(st):
                    """e2p mult + outer product + chunked store."""
                    args, fc, base, f = st["args"], st["fc"], st["base"], st["f"]
                    fq = f // N_OUT_CHUNKS
                    eq3 = args[:, Z_DIM * f : 13 * f].rearrange(
                        "p (a f) -> p a f", a=A_DIM
                    )
                    fc_b = fc.unsqueeze(1).broadcast_to((128, A_DIM, f))
                    (nc.gpsimd if GPS_E2P else nc.vector).tensor_mul(eq3, eq3, fc_b)

                    ez3 = args[:, 0 : Z_DIM * f].rearrange("p (z f) -> p z f", z=Z_DIM)
                    for c in range(N_OUT_CHUNKS):
                        f0 = c * fq
                        outc = pool.tile(
                            [128, fq * OUT_D], out_dt, tag="outc", bufs=OUTC_BUFS,
                            name=f"outc{base}_{c}",
                        )
                        outc3 = outc.rearrange("p (f az) -> p f az", az=OUT_D)
                        e1s = ez3[:, :, f0 : f0 + fq].transpose([0, 2, 1])
                        oeng = nc.gpsimd if c < GPS_OUTER else nc.vector
                        if OUTER4D:
                            out4 = outc.rearrange(
                                "p (f a z) -> p a f z", a=A_DIM, z=Z_DIM
                            )
                            e1b = e1s.unsqueeze(1).broadcast_to(
                                (128, A_DIM, fq, Z_DIM)
                            )
                            e2b = (
                                eq3[:, :, f0 : f0 + fq]
                                .unsqueeze(3)
                                .broadcast_to((128, A_DIM, fq, Z_DIM))
                            )
                            oeng.tensor_mul(out4, e1b, e2b)
                        else:
                            for a in range(A_DIM):
                                e2s = (
                                    eq3[:, a, f0 : f0 + fq]
                                    .unsqueeze(2)
                                    .broadcast_to((128, fq, Z_DIM))
                                )
                                oeng.tensor_mul(
                                    outc3[:, :, a * Z_DIM : (a + 1) * Z_DIM], e1s, e2s
                                )
                        nc.scalar.dma_start(
                            bass.AP(
                                out_d,
                                OUT_D * (base + f0),
                                [[OUT_D * f, 128], [1, OUT_D * fq]],
                            ),
                            outc,
                        )

                from collections import deque

                # tile plan: optionally split leading (and trailing) tiles
                # to shorten pipeline ramp (and drain)
                if TAPER == 1 and ntiles >= 2:
                    sizes = (
                        [f // 2, f // 2]
                        + [f] * (ntiles - 2)
                        + [f // 2, f // 2]
                    )
                elif TAPER == 2 and ntiles >= 2:
                    sizes = [f // 2, f // 2] + [f] * (ntiles - 1)
                elif TAPER == 3 and ntiles >= 2:
                    sizes = [f // 4] * 4 + [f] * (ntiles - 1)
                elif TAPER == 4 and ntiles >= 2:
                    sizes = [f // 4] * 4 + [f] * (ntiles - 2) + [f // 2, f // 2]
                elif TAPER == 5 and ntiles >= 2:
                    sizes = [f // 4, f // 4, f // 2] + [f] * (ntiles - 1)
                elif TAPER == 6 and ntiles >= 3:
                    sizes = [f // 2, f // 2, f // 2, f // 2] + [f] * (ntiles - 2)
                elif TAPER == 7 and ntiles >= 3:
                    sizes = (
                        [f // 4, f // 4, f // 2, f // 2, f // 2]
                        + [f] * (ntiles - 2)
                    )
                else:
                    sizes = [f] * ntiles
                plan = []
                total = 0
                for fs in sizes:
                    plan.append((total, fs))
                    total += 128 * fs
                assert total == 128 * f * ntiles, (total, sizes)

                for _rep in range(repeat):
                    pending = deque()
                    for i, (b, fs) in enumerate(plan):
                        pending.append(front(i, b, fs))
                        if len(pending) > PIPE_DEPTH:
                            back(pending.popleft())
                    while pending:
                        back(pending.popleft())
                if repeat > 1:
                    nc.sync.dma_start(tok_d.ap(), fb)

    nc.compile()
    return nc


_NC_CACHE: dict = {}


def _get_nc(pc: int, f: int, repeat: int = 1):
    key = (pc, f, repeat)
    if key not in _NC_CACHE:
        _NC_CACHE[key] = build_nc(pc, f, repeat)
    return _NC_CACHE[key]


def _make_const_inputs(EtaA, ShfA, Gamma, ShfZ):
    sg = np.sqrt(np.asarray(Gamma, np.float64))            # (9,)
    cz = np.cos(np.asarray(ShfZ, np.float64))              # (9,)
    se = math.sqrt(float(np.asarray(EtaA).reshape(-1)[0]))
    rscale = np.broadcast_to(sg, (128, Z_DIM)).astype(np.float32)
    rbias = np.broadcast_to(-sg * cz, (128, Z_DIM)).astype(np.float32)
    qscale = np.full((128, 1), 0.5 * se, np.float32)
    qbias = np.broadcast_to(
        -se * np.asarray(ShfA, np.float64), (128, A_DIM)
    ).astype(np.float32)
    fbias = np.full((128, 1), -math.sqrt(2.0), np.float32)
    return (
        np.ascontiguousarray(rscale),
        np.ascontiguousarray(rbias),
        qscale,
        np.ascontiguousarray(qbias),
        fbias,
    )


_LAST_RESULT = None  # BassKernelResults of the most recent run (for test harness)


def _prepare(vectors12, EtaA, ShfA, Gamma, ShfZ, pc, f, n_cores, repeat=1):
    v = np.ascontiguousarray(np.asarray(vectors12, np.float32))
    rscale, rbias, qscale, qbias, fbias = _make_const_inputs(EtaA, ShfA, Gamma, ShfZ)
    nc = _get_nc(pc, f, repeat)
    in_maps = []
    for c in range(n_cores):
        in_maps.append(
            {
                "vectors12": np.ascontiguousarray(v[:, c * pc : (c + 1) * pc, :]),
                "rscale": rscale,
                "rbias": rbias,
                "qscale": qscale,
                "qbias": qbias,
                "fbias": fbias,
            }
        )
    return nc, in_maps


def _run(vectors12, EtaA, ShfA, Gamma, ShfZ, pc, f, n_cores):
    global _LAST_RESULT
    nc, in_maps = _prepare(vectors12, EtaA, ShfA, Gamma, ShfZ, pc, f, n_cores)
    res = run_bass_kernel_spmd(nc, in_maps, core_ids=list(range(n_cores)))
    _LAST_RESULT = res
    out = np.concatenate([res.results[c]["out"] for c in range(n_cores)], axis=0)
    if out.dtype != np.float32:
        out = out.astype(np.float32)
    return out


def kernel(vectors12, EtaA, ShfA, Gamma, ShfZ):
    return _run(vectors12, EtaA, ShfA, Gamma, ShfZ, PC, F, N_CORES)

